# revision 16
# baseline (speedup 1.0000x reference)
# Lagrangian-NN qddot kernel for TRN2 (8 NeuronCores, data-parallel over batch).
#
# Math: scalar L(q,qdot) = MLP(24->256x4->1, softplus). Per sample:
#   M = d2L/dqdot2 + 0.01 I ; C = d2L/dqdot dq ; qddot = M^-1 (dL/dq - C qdot).
# Batched fwd+bwd gives grad; 12 qdot-direction forward-over-reverse tangents give
# H[:,12:] whose symmetry supplies both M and the Coriolis contraction.
# Everything except PSUM accumulation and the H/solve stage runs in fp16:
#  - DVE elementwise ops are emitted as scalar_tensor_tensor/tensor_scalar
#    (InstTensorScalarPtr) which support the 2x/4x DVE perf modes on all-SBUF
#    2-byte operands; PSUM results are staged to fp16 SBUF via Act/Pool copies.
#  - softplus/sigmoid composed from {exp,ln} + DVE max/adds (single ACT table):
#    Z = max(A,0) + ln(exp(-|A|)+1) ; S = exp(A - Z).
#  - M = 0.01(I + 100*Hqd) with ||100*Hqd|| <= 0.035, so the 12x12 solve is a
#    3-term Neumann series, fused over all 8 sample groups per core.
import os
import sys
import numpy as np

for p in ("/opt/trn_rl_repo", "/root/.axon_site/_ro/trn_rl_repo"):
    if p not in sys.path:
        sys.path.insert(0, p)

import concourse.bass as bass
import concourse.mybir as mybir
import concourse.tile as tile
from concourse import bacc
from concourse.bass_utils import run_bass_kernel_spmd

F32 = mybir.dt.float32
F16 = mybir.dt.float16
AF = mybir.ActivationFunctionType
ALU = mybir.AluOpType
AX = mybir.AxisListType

B, ND, H, NC = 8192, 12, 256, 8
N = B // NC          # samples per core
IN = 2 * ND          # 24
T = 64               # samples per tangent block
NT = N // T          # 16 blocks
NG = N // 128        # 8 groups of 128 samples
FD = ND * T          # 768 tangent free dim
CH = 512             # psum bank chunk (fp32 cols)
KT = H // 128        # 2 k-tiles per hidden dim

_cache = {}


def build_kernel():
    nc = bacc.Bacc("TRN2", target_bir_lowering=False)
    dx16 = nc.dram_tensor("x16", (N, IN), F16, kind="ExternalInput")
    dqd = nc.dram_tensor("qd32", (N, ND), F32, kind="ExternalInput")
    dwt0 = nc.dram_tensor("wt0", (IN, H), F16, kind="ExternalInput")
    dwt = {l: nc.dram_tensor(f"wt{l}", (H, H), F16, kind="ExternalInput")
           for l in (1, 2, 3)}
    dwn = {l: nc.dram_tensor(f"wn{l}", (H, H), F16, kind="ExternalInput")
           for l in (1, 2, 3)}
    dw0n = nc.dram_tensor("w0n", (H, IN), F16, kind="ExternalInput")
    dw0qr = nc.dram_tensor("w0qr", (H, FD), F16, kind="ExternalInput")
    dbs = [nc.dram_tensor(f"b{l}", (H, 1), F32, kind="ExternalInput") for l in range(4)]
    dw4 = nc.dram_tensor("w4", (H, 1), F32, kind="ExternalInput")
    did16 = nc.dram_tensor("id16", (128, 128), F16, kind="ExternalInput")
    did32 = nc.dram_tensor("id32", (128, 128), F32, kind="ExternalInput")
    dout = nc.dram_tensor("qdd", (N, ND), F32, kind="ExternalOutput")

    with tile.TileContext(nc) as tc:
        with tc.tile_pool(name="wp", bufs=1) as wp, \
             tc.tile_pool(name="ap", bufs=1) as ap, \
             tc.tile_pool(name="sc", bufs=2) as sc, \
             tc.tile_pool(name="tg", bufs=2) as tg, \
             tc.tile_pool(name="hp", bufs=1) as hp, \
             tc.tile_pool(name="psB", bufs=3, space="PSUM") as psB, \
             tc.tile_pool(name="psT", bufs=2, space="PSUM") as psT:

            # ---- pin ONE activation table (exp+ln+abs+identity+copy) so the
            # compiler's greedy per-func chooser doesn't thrash table loads.
            from concourse.hw_specs import get_activation_tables
            need = {AF.Exp, AF.Ln, AF.Abs, AF.Identity, AF.Copy}
            set_id = next(i for i, (_, fns) in
                          enumerate(get_activation_tables(nc.m.arch).items())
                          if need <= fns)
            nc.scalar.add_instruction(mybir.InstLoadActFuncSet(
                name=nc.get_next_instruction_name(), act_func_set_id=set_id,
                ins=[], outs=[]))

            # ---- weight / const loads --------------------------------------
            id16 = wp.tile([128, 128], F16)
            nc.sync.dma_start(id16[:], did16[:])
            id32 = wp.tile([128, 128], F32)
            nc.sync.dma_start(id32[:], did32[:])

            WT0 = wp.tile([IN, H], F16)
            nc.sync.dma_start(WT0[:], dwt0[:])

            def load2(dram, Fr, tag, dt=F16):
                parts = []
                for ki in range(KT):
                    t_ = wp.tile([128, Fr], dt, tag=f"{tag}{ki}")
                    nc.sync.dma_start(t_[:], dram[ki * 128:(ki + 1) * 128, :])
                    parts.append(t_)
                return parts

            WT = {l: load2(dwt[l], H, f"wt{l}_") for l in (1, 2, 3)}
            Wn = {l: load2(dwn[l], H, f"wn{l}_") for l in (1, 2, 3)}
            W0n = load2(dw0n, IN, "w0n_")
            W0qr = load2(dw0qr, FD, "w0qr_")
            bs = []
            for l in range(4):
                row = []
                for ki in range(KT):
                    t_ = wp.tile([128, 1], F32, tag=f"b{l}_{ki}")
                    nc.sync.dma_start(t_[:], dbs[l][ki * 128:(ki + 1) * 128, :])
                    row.append(t_)
                bs.append(row)
            w4t = []
            for ki in range(KT):
                t_ = wp.tile([128, 1], F32, tag=f"w4_{ki}")
                nc.sync.dma_start(t_[:], dw4[ki * 128:(ki + 1) * 128, :])
                w4t.append(t_)

            XS = hp.tile([128, NG, IN], F16)
            qd_all = hp.tile([128, NG, ND], F32)
            for g in range(NG):
                nc.sync.dma_start(XS[:, g, :], dx16[g * 128:(g + 1) * 128, :])
                nc.sync.dma_start(qd_all[:, g, :], dqd[g * 128:(g + 1) * 128, :])

            # ---- XT = X^T [24, N] fp16 -------------------------------------
            XT = hp.tile([IN, N], F16)
            for g in range(NG):
                pt = psT.tile([IN, 128], F16, tag="pt")
                nc.tensor.transpose(pt[:], XS[:, g, :], id16[:])
                nc.vector.tensor_copy(XT[:, g * 128:(g + 1) * 128], pt[:])

            def mm(ps_ap, lhsT_list, rhs_list, Fr):
                nk = len(lhsT_list)
                for c0 in range(0, Fr, CH):
                    ce = min(Fr, c0 + CH)
                    for ki in range(nk):
                        nc.tensor.matmul(ps_ap[:, c0:ce], lhsT_list[ki],
                                         rhs_list[ki][:, c0:ce],
                                         start=(ki == 0), stop=(ki == nk - 1))

            # ---- forward: Z chain + S (softplus/sigmoid via exp/ln) --------
            S = {}
            Zprev = [XT[:]]
            lhs0 = [WT0[:]]
            for l in range(4):
                Zcur = []
                for ot in range(KT):
                    ps = psB.tile([128, 1024], F32, tag="mm")
                    if l == 0:
                        mm(ps[:, 0:N], [lhs0[0][:, ot * 128:(ot + 1) * 128]],
                           Zprev, N)
                    else:
                        lts = [WT[l][ki][:, ot * 128:(ot + 1) * 128] for ki in range(KT)]
                        mm(ps[:, 0:N], lts, Zprev, N)
                    A16 = sc.tile([128, N], F16, tag="A16")
                    nc.scalar.activation(A16[:], ps[0:128, 0:N], AF.Identity,
                                         bias=bs[l][ot][:])
                    ab = sc.tile([128, N], F16, tag="t1")
                    nc.scalar.activation(ab[:], A16[:], AF.Abs)
                    ex = sc.tile([128, N], F16, tag="t2")
                    nc.scalar.activation(ex[:], ab[:], AF.Exp, scale=-1.0)
                    Ln = sc.tile([128, N], F16, tag="L")
                    nc.scalar.activation(Ln[:], ex[:], AF.Ln, bias=1.0)
                    rl = sc.tile([128, N], F16, tag="t1")
                    nc.vector.tensor_scalar_max(rl[:], A16[:], 0.0)
                    Z = sc.tile([128, N], F16, tag="Z", bufs=3)
                    nc.vector.tensor_add(Z[:], rl[:], Ln[:])
                    d = sc.tile([128, N], F16, tag="t2")
                    nc.vector.tensor_sub(d[:], A16[:], Z[:])
                    St = ap.tile([128, N], F16, tag=f"S{l}_{ot}")
                    nc.scalar.activation(St[:], d[:], AF.Exp)
                    S[(l, ot)] = St
                    Zcur.append(Z)
                Zprev = [z[:] for z in Zcur]

            # ---- D4 / c4 ----------------------------------------------------
            D4, c4 = [], []
            for ot in range(KT):
                Dt4 = ap.tile([128, N], F16, tag=f"D4_{ot}")
                nc.vector.tensor_scalar_mul(Dt4[:], S[(3, ot)][:], w4t[ot][:])
                D4.append(Dt4)
                OmS = sc.tile([128, N], F16, tag="OmS")
                nc.vector.tensor_scalar(OmS[:], S[(3, ot)][:], -1.0, 1.0,
                                        ALU.mult, ALU.add)
                tm = sc.tile([128, N], F16, tag="t1")
                nc.vector.tensor_mul(tm[:], OmS[:], S[(3, ot)][:])
                ct = ap.tile([128, N], F16, tag=f"c4_{ot}")
                nc.vector.tensor_scalar_mul(ct[:], tm[:], w4t[ot][:])
                c4.append(ct)

            # ---- backward D chain + F/E + g_q ------------------------------
            F = {}
            E1 = []
            Dprev = [d[:] for d in D4]
            for l in (2, 1, 0):
                Dcur = []
                for ot in range(KT):
                    ps = psB.tile([128, 1024], F32, tag="mm")
                    lts = [Wn[l + 1][ki][:, ot * 128:(ot + 1) * 128] for ki in range(KT)]
                    mm(ps[:, 0:N], lts, Dprev, N)
                    Ucp = sc.tile([128, N], F16, tag="Ucp")
                    nc.scalar.activation(Ucp[:], ps[0:128, 0:N], AF.Copy)
                    Dt = sc.tile([128, N], F16, tag="Dt", bufs=3)
                    nc.vector.tensor_mul(Dt[:], Ucp[:], S[(l, ot)][:])
                    if l > 0:
                        Ft = ap.tile([128, N], F16, tag=f"F{l}_{ot}")
                        nc.vector.tensor_sub(Ft[:], Ucp[:], Dt[:])
                        F[(l, ot)] = Ft
                    else:
                        OmS1 = sc.tile([128, N], F16, tag="OmS")
                        nc.vector.tensor_scalar(OmS1[:], S[(0, ot)][:], -1.0, 1.0,
                                                ALU.mult, ALU.add)
                        Et = ap.tile([128, N], F16, tag=f"E1_{ot}")
                        nc.vector.tensor_mul(Et[:], Dt[:], OmS1[:])
                        E1.append(Et)
                    Dcur.append(Dt)
                Dprev = [d[:] for d in Dcur]

            psG = psB.tile([128, 1024], F32, tag="mm")
            mm(psG[0:IN, 0:N], [W0n[ki][:] for ki in range(KT)], Dprev, N)
            Gcp = hp.tile([IN, N], F32)
            nc.scalar.activation(Gcp[:], psG[0:IN, 0:N], AF.Copy)

            # transposed g_q per group -> gqT [128, g, 12]
            gqT = hp.tile([128, NG, ND], F32)
            for g in range(NG):
                ptg = psT.tile([128, ND], F32, tag="pt")
                nc.tensor.transpose(ptg[:], Gcp[0:ND, g * 128:(g + 1) * 128],
                                    id32[0:ND, 0:ND])
                nc.vector.tensor_copy(gqT[:, g, :], ptg[:])

            # ---- tangent blocks --------------------------------------------
            Hq = hp.tile([128, NG, ND, ND], F32)   # H[x_j<12, qd_i] -> [p,g,i,j]
            Hm = hp.tile([128, NG, ND, ND], F32)   # 100*H[x_12+j, qd_i]
            Hc = None

            def bca(l, ot, sl):
                return S[(l, ot)][:, sl].unsqueeze(1).broadcast_to((128, ND, T))

            def flat(ts):
                return [t_[:].rearrange("p d t -> p (d t)") for t_ in ts]

            def make_steps(b, Hc_ref):
                """Return the tangent-chain step closures for block b.

                Blocks are emitted pairwise-interleaved so every engine queue
                alternates between two independent chains (fills the bubbles a
                single serial chain leaves on the other engines)."""
                i = b & 1
                off = i * T
                g = b // 2
                sl = slice(b * T, (b + 1) * T)
                st = {}

                def w0v(ot):
                    return W0qr[ot][:].rearrange("p (d t) -> p d t", d=ND)

                def tgt(tagbase, **kw):
                    kw.setdefault("bufs", 1)
                    return tg.tile([128, ND, T], F16, tag=f"{tagbase}_{i}",
                                   name=tagbase, **kw)

                def s_zd1():
                    st["Zd1"] = []
                    for ot in range(KT):
                        z = tgt(f"Zd1_{ot}")
                        nc.vector.tensor_mul(z[:], w0v(ot), bca(0, ot, sl))
                        st["Zd1"].append(z)

                def mk_mm(src_key, lW, dst_key, rows=128):
                    def s_mm():
                        pss = []
                        for ot in range(KT if rows == 128 else 1):
                            ps = psB.tile([128, 1024], F32, tag="mm", name="ps")
                            if rows == 128:
                                lts = [lW[ki][:, ot * 128:(ot + 1) * 128]
                                       for ki in range(KT)]
                            else:
                                lts = [lW[ki][:] for ki in range(KT)]
                            mm(ps[0:rows, 0:FD], lts, flat(st[src_key]), FD)
                            pss.append(ps)
                        st[dst_key] = pss
                    return s_mm

                def mk_copy(ps_key, dst_key, dtag):
                    def s_copy():
                        st[dst_key] = []
                        for ot in range(KT):
                            cc = tgt(f"{dtag}_{ot}")
                            nc.scalar.activation(cc[:].rearrange("p d t -> p (d t)"),
                                                 st[ps_key][ot][0:128, 0:FD], AF.Copy)
                            st[dst_key].append(cc)
                    return s_copy

                def mk_mul(in_key, coefs, dst_key, dtag=None):
                    def s_mul():
                        st[dst_key] = []
                        for ot in range(KT):
                            z = tgt(f"{dtag or dst_key}_{ot}")
                            nc.vector.tensor_mul(
                                z[:], st[in_key][ot][:],
                                coefs[ot][:, sl].unsqueeze(1).broadcast_to(
                                    (128, ND, T)))
                            st[dst_key].append(z)
                    return s_mul

                def mk_umul_direct(ps_key, coefs, dst_key):
                    def s_mul():
                        st[dst_key] = []
                        for ot in range(KT):
                            z = tgt(f"u2_{ot}")
                            psv = st[ps_key][ot][0:128, 0:FD].rearrange(
                                "p (d t) -> p d t", d=ND)
                            nc.vector.tensor_mul(
                                z[:], psv,
                                coefs[ot][:, sl].unsqueeze(1).broadcast_to(
                                    (128, ND, T)))
                            st[dst_key].append(z)
                    return s_mul

                def mk_tmul(zd_key, coefs, dst_key, use_w0=False):
                    def s_mul():
                        st[dst_key] = []
                        for ot in range(KT):
                            z = tgt(f"t_{ot}")
                            src = w0v(ot) if use_w0 else st[zd_key][ot][:]
                            nc.vector.tensor_mul(
                                z[:], src,
                                coefs[ot][:, sl].unsqueeze(1).broadcast_to(
                                    (128, ND, T)))
                            st[dst_key].append(z)
                    return s_mul

                def mk_add(u_key, t_key, dst_key, dtag, pool_mask=3):
                    def s_add():
                        st[dst_key] = []
                        for ot in range(KT):
                            dd = tgt(f"{dtag}_{ot}")
                            if (pool_mask >> ot) & 1:
                                nc.gpsimd.tensor_add(
                                    dd[:].rearrange("p d t -> p (d t)"),
                                    st[u_key][ot][:].rearrange("p d t -> p (d t)"),
                                    st[t_key][ot][:].rearrange("p d t -> p (d t)"))
                            else:
                                nc.vector.tensor_add(dd[:], st[u_key][ot][:],
                                                     st[t_key][ot][:])
                            st[dst_key].append(dd)
                    return s_add

                def s_hc():
                    if off == 0:
                        Hc_ref[0] = hp.tile([IN, ND, 128], F16, tag="Hc", bufs=2,
                                            name="Hc")
                    nc.scalar.activation(
                        Hc_ref[0][:, :, off:off + T],
                        st["psH"][0][0:IN, 0:FD].rearrange("p (d t) -> p d t", d=ND),
                        AF.Copy)

                def s_hstage():
                    if off != T:
                        return
                    ptH = psT.tile([128, 288], F16, tag="pt", name="ptH")
                    for dcol in range(ND):
                        nc.tensor.transpose(ptH[:, dcol * IN:(dcol + 1) * IN],
                                            Hc_ref[0][:, dcol, :], id16[0:IN, 0:IN])
                    ptHv = ptH[:, 0:ND * IN].rearrange("p (d k) -> p d k", d=ND)
                    nc.vector.tensor_copy(Hq[:, g, :, :], ptHv[:, :, 0:ND])
                    nc.vector.tensor_scalar_mul(Hm[:, g, :, :], ptHv[:, :, ND:IN],
                                                100.0)

                S2c = [S[(1, 0)], S[(1, 1)]]
                S3c = [S[(2, 0)], S[(2, 1)]]
                F3c = [F[(2, 0)], F[(2, 1)]]
                F2c = [F[(1, 0)], F[(1, 1)]]
                S1c = [S[(0, 0)], S[(0, 1)]]
                return [
                    s_zd1,
                    mk_mm("Zd1", WT[1], "psA"),
                    mk_copy("psA", "c2", "c"),
                    mk_mul("c2", S2c, "Zd2"),
                    mk_mm("Zd2", WT[2], "psB"),
                    mk_copy("psB", "c3", "c"),
                    mk_mul("c3", S3c, "Zd3"),
                    mk_mm("Zd3", WT[3], "psC"),
                    mk_copy("psC", "cY", "c"),
                    mk_mul("cY", c4, "Dd4"),
                    mk_mm("Dd4", Wn[3], "psY3"),
                    mk_copy("psY3", "y3", "y"),
                    mk_mul("y3", S3c, "u3", dtag="u"),
                    mk_tmul("Zd3", F3c, "t3"),
                    mk_add("u3", "t3", "Dd3", "DdA"),
                    mk_mm("Dd3", Wn[2], "psY2"),
                    mk_umul_direct("psY2", S2c, "u2"),
                    mk_tmul("Zd2", F2c, "t2"),
                    mk_add("u2", "t2", "Dd2", "DdB"),
                    mk_mm("Dd2", Wn[1], "psY1"),
                    mk_copy("psY1", "y1", "y"),
                    mk_mul("y1", S1c, "u1", dtag="u"),
                    mk_tmul(None, E1, "t1", use_w0=True),
                    mk_add("u1", "t1", "Dd1", "DdA", pool_mask=1),
                    mk_mm("Dd1", W0n, "psH", rows=IN),
                    s_hc,
                    s_hstage,
                ]

            Hc_refs = [[None] for _ in range(NT // 2)]
            steps_all = [make_steps(b, Hc_refs[b // 2]) for b in range(NT)]
            SKEW = 14
            nsteps = len(steps_all[0])
            for tick in range(nsteps + SKEW * (NT - 1) + 1):
                for b in range(NT):
                    j = tick - SKEW * b
                    if 0 <= j < nsteps:
                        steps_all[b][j]()

            # ---- coriolis + rhs + Neumann solve (all groups fused) ---------
            prod = hp.tile([128, NG, ND, ND], F32, tag="prod", bufs=2)
            nc.vector.tensor_tensor(
                prod[:], Hq[:],
                qd_all[:].unsqueeze(2).broadcast_to((128, NG, ND, ND)), ALU.mult)
            cor = hp.tile([128, NG, ND], F32)
            nc.vector.tensor_reduce(cor[:].unsqueeze(3), prod[:], op=ALU.add, axis=AX.X)
            r = hp.tile([128, NG, ND], F32)
            nc.vector.scalar_tensor_tensor(r[:], cor[:], -1.0, gqT[:],
                                           ALU.mult, ALU.add)
            z = hp.tile([128, NG, ND], F32, tag="z", bufs=2)
            nc.vector.tensor_copy(z[:], r[:])
            for _ in range(3):
                pr = hp.tile([128, NG, ND, ND], F32, tag="prod", bufs=2)
                nc.vector.tensor_tensor(
                    pr[:], Hm[:],
                    z[:].unsqueeze(2).broadcast_to((128, NG, ND, ND)), ALU.mult)
                s_ = hp.tile([128, NG, ND], F32, tag="s", bufs=2)
                nc.vector.tensor_reduce(s_[:].unsqueeze(3), pr[:], op=ALU.add, axis=AX.X)
                zn = hp.tile([128, NG, ND], F32, tag="z", bufs=2)
                nc.vector.scalar_tensor_tensor(zn[:], s_[:], -1.0, r[:],
                                               ALU.mult, ALU.add)
                z = zn
            o = hp.tile([128, NG, ND], F32)
            nc.vector.tensor_scalar_mul(o[:], z[:], 100.0)
            for g in range(NG):
                nc.sync.dma_start(dout[g * 128:(g + 1) * 128, :], o[:, g, :])

    nc.compile()
    return nc


def kernel(**inputs):
    f16 = np.float16
    f32 = np.float32
    q = np.asarray(inputs["q"], f32)
    qdot = np.asarray(inputs["qdot"], f32)
    if "nc" not in _cache:
        _cache["nc"] = build_kernel()
    nc = _cache["nc"]
    W = [np.asarray(inputs[f"W{i}"], f32) for i in range(5)]
    X16 = np.ascontiguousarray(np.concatenate([q, qdot], axis=1)).astype(f16)
    base = {
        "wt0": np.ascontiguousarray(W[0].T).astype(f16),
        "wt1": np.ascontiguousarray(W[1].T).astype(f16),
        "wt2": np.ascontiguousarray(W[2].T).astype(f16),
        "wt3": np.ascontiguousarray(W[3].T).astype(f16),
        "wn1": np.ascontiguousarray(W[1]).astype(f16),
        "wn2": np.ascontiguousarray(W[2]).astype(f16),
        "wn3": np.ascontiguousarray(W[3]).astype(f16),
        "w0n": np.ascontiguousarray(W[0]).astype(f16),
        "w0qr": np.ascontiguousarray(
            np.repeat(W[0][:, ND:].astype(f16), T, axis=1)),
        "b0": inputs["b0"].reshape(H, 1).astype(f32),
        "b1": inputs["b1"].reshape(H, 1).astype(f32),
        "b2": inputs["b2"].reshape(H, 1).astype(f32),
        "b3": inputs["b3"].reshape(H, 1).astype(f32),
        "w4": np.ascontiguousarray(W[4].reshape(H, 1)).astype(f32),
        "id16": np.eye(128, dtype=f16),
        "id32": np.eye(128, dtype=f32),
    }
    in_maps = []
    for c in range(NC):
        m = dict(base)
        m["x16"] = X16[c * N:(c + 1) * N]
        m["qd32"] = np.ascontiguousarray(qdot[c * N:(c + 1) * N])
        in_maps.append(m)
    res = run_bass_kernel_spmd(nc, in_maps, core_ids=list(range(NC)),
                               trace=bool(os.environ.get("LNN_TRACE")))
    _cache["last"] = res
    out = np.concatenate([res.results[c]["qdd"] for c in range(NC)], axis=0)
    return out.astype(f32)


# revision 18
# speedup vs baseline: 1.1127x; 1.1127x over previous
# Lagrangian-NN qddot kernel for TRN2 (8 NeuronCores, data-parallel over batch).
#
# Math: scalar L(q,qdot) = MLP(24->256x4->1, softplus). Per sample:
#   M = d2L/dqdot2 + 0.01 I ; C = d2L/dqdot dq ; qddot = M^-1 (dL/dq - C qdot).
# Batched fwd+bwd gives grad; 12 qdot-direction forward-over-reverse tangents give
# H[:,12:] whose symmetry supplies both M and the Coriolis contraction.
# Everything except PSUM accumulation and the H/solve stage runs in fp16:
#  - DVE elementwise ops are emitted as scalar_tensor_tensor/tensor_scalar
#    (InstTensorScalarPtr) which support the 2x/4x DVE perf modes on all-SBUF
#    2-byte operands; PSUM results are staged to fp16 SBUF via Act/Pool copies.
#  - softplus/sigmoid composed from {exp,ln} + DVE max/adds (single ACT table):
#    Z = max(A,0) + ln(exp(-|A|)+1) ; S = exp(A - Z).
#  - M = 0.01(I + 100*Hqd) with ||100*Hqd|| <= 0.035, so the 12x12 solve is a
#    3-term Neumann series, fused over all 8 sample groups per core.
import os
import sys
import numpy as np

for p in ("/opt/trn_rl_repo", "/root/.axon_site/_ro/trn_rl_repo"):
    if p not in sys.path:
        sys.path.insert(0, p)

import concourse.bass as bass
import concourse.mybir as mybir
import concourse.tile as tile
from concourse import bacc
from concourse.bass_utils import run_bass_kernel_spmd

F32 = mybir.dt.float32
F16 = mybir.dt.float16
AF = mybir.ActivationFunctionType
ALU = mybir.AluOpType
AX = mybir.AxisListType

B, ND, H, NC = 8192, 12, 256, 8
N = B // NC          # samples per core
IN = 2 * ND          # 24
T = 32               # samples per tangent block (FD=384 -> 1 psum bank)
NT = N // T          # 32 blocks
NG = N // 128        # 8 groups of 128 samples
FD = ND * T          # 768 tangent free dim
CH = 512             # psum bank chunk (fp32 cols)
KT = H // 128        # 2 k-tiles per hidden dim

_cache = {}


def build_kernel():
    nc = bacc.Bacc("TRN2", target_bir_lowering=False)
    dx16 = nc.dram_tensor("x16", (N, IN), F16, kind="ExternalInput")
    dqd = nc.dram_tensor("qd32", (N, ND), F32, kind="ExternalInput")
    dwt0 = nc.dram_tensor("wt0", (IN, H), F16, kind="ExternalInput")
    dwt = {l: nc.dram_tensor(f"wt{l}", (H, H), F16, kind="ExternalInput")
           for l in (1, 2, 3)}
    dwn = {l: nc.dram_tensor(f"wn{l}", (H, H), F16, kind="ExternalInput")
           for l in (1, 2, 3)}
    dw0n = nc.dram_tensor("w0n", (H, IN), F16, kind="ExternalInput")
    dw0qr = nc.dram_tensor("w0qr", (H, FD), F16, kind="ExternalInput")
    dbs = [nc.dram_tensor(f"b{l}", (H, 1), F32, kind="ExternalInput") for l in range(4)]
    dw4 = nc.dram_tensor("w4", (H, 1), F32, kind="ExternalInput")
    did16 = nc.dram_tensor("id16", (128, 128), F16, kind="ExternalInput")
    did32 = nc.dram_tensor("id32", (128, 128), F32, kind="ExternalInput")
    dout = nc.dram_tensor("qdd", (N, ND), F32, kind="ExternalOutput")

    with tile.TileContext(nc) as tc:
        with tc.tile_pool(name="wp", bufs=1) as wp, \
             tc.tile_pool(name="ap", bufs=1) as ap, \
             tc.tile_pool(name="sc", bufs=2) as sc, \
             tc.tile_pool(name="tg", bufs=2) as tg, \
             tc.tile_pool(name="hp", bufs=1) as hp, \
             tc.tile_pool(name="psB", bufs=6, space="PSUM") as psB, \
             tc.tile_pool(name="psT", bufs=2, space="PSUM") as psT:

            # ---- pin ONE activation table (exp+ln+abs+identity+copy) so the
            # compiler's greedy per-func chooser doesn't thrash table loads.
            from concourse.hw_specs import get_activation_tables
            need = {AF.Exp, AF.Ln, AF.Abs, AF.Identity, AF.Copy}
            set_id = next(i for i, (_, fns) in
                          enumerate(get_activation_tables(nc.m.arch).items())
                          if need <= fns)
            nc.scalar.add_instruction(mybir.InstLoadActFuncSet(
                name=nc.get_next_instruction_name(), act_func_set_id=set_id,
                ins=[], outs=[]))

            # ---- weight / const loads --------------------------------------
            id16 = wp.tile([128, 128], F16)
            nc.sync.dma_start(id16[:], did16[:])
            id32 = wp.tile([128, 128], F32)
            nc.sync.dma_start(id32[:], did32[:])

            WT0 = wp.tile([IN, H], F16)
            nc.sync.dma_start(WT0[:], dwt0[:])

            def load2(dram, Fr, tag, dt=F16):
                parts = []
                for ki in range(KT):
                    t_ = wp.tile([128, Fr], dt, tag=f"{tag}{ki}")
                    nc.sync.dma_start(t_[:], dram[ki * 128:(ki + 1) * 128, :])
                    parts.append(t_)
                return parts

            WT = {l: load2(dwt[l], H, f"wt{l}_") for l in (1, 2, 3)}
            Wn = {l: load2(dwn[l], H, f"wn{l}_") for l in (1, 2, 3)}
            W0n = load2(dw0n, IN, "w0n_")
            W0qr = load2(dw0qr, FD, "w0qr_")
            bs = []
            for l in range(4):
                row = []
                for ki in range(KT):
                    t_ = wp.tile([128, 1], F32, tag=f"b{l}_{ki}")
                    nc.sync.dma_start(t_[:], dbs[l][ki * 128:(ki + 1) * 128, :])
                    row.append(t_)
                bs.append(row)
            w4t = []
            for ki in range(KT):
                t_ = wp.tile([128, 1], F32, tag=f"w4_{ki}")
                nc.sync.dma_start(t_[:], dw4[ki * 128:(ki + 1) * 128, :])
                w4t.append(t_)

            XS = hp.tile([128, NG, IN], F16)
            qd_all = hp.tile([128, NG, ND], F32)
            for g in range(NG):
                nc.sync.dma_start(XS[:, g, :], dx16[g * 128:(g + 1) * 128, :])
                nc.sync.dma_start(qd_all[:, g, :], dqd[g * 128:(g + 1) * 128, :])

            # ---- XT = X^T [24, N] fp16 -------------------------------------
            XT = hp.tile([IN, N], F16)
            for g in range(NG):
                pt = psT.tile([IN, 128], F16, tag="pt")
                nc.tensor.transpose(pt[:], XS[:, g, :], id16[:])
                nc.vector.tensor_copy(XT[:, g * 128:(g + 1) * 128], pt[:])

            def mm(ps_ap, lhsT_list, rhs_list, Fr):
                nk = len(lhsT_list)
                for c0 in range(0, Fr, CH):
                    ce = min(Fr, c0 + CH)
                    for ki in range(nk):
                        nc.tensor.matmul(ps_ap[:, c0:ce], lhsT_list[ki],
                                         rhs_list[ki][:, c0:ce],
                                         start=(ki == 0), stop=(ki == nk - 1))

            # ---- forward: Z chain + S (softplus/sigmoid via exp/ln) --------
            S = {}
            Zprev = [XT[:]]
            lhs0 = [WT0[:]]
            for l in range(4):
                Zcur = []
                for ot in range(KT):
                    if l == 0:
                        lts = [lhs0[0][:, ot * 128:(ot + 1) * 128]]
                    else:
                        lts = [WT[l][ki][:, ot * 128:(ot + 1) * 128] for ki in range(KT)]
                    A16 = sc.tile([128, N], F16, tag="A16")
                    for h in range(2):
                        slh = slice(h * CH, (h + 1) * CH)
                        ps = psB.tile([128, CH], F32, tag="mm")
                        mm(ps[:], lts, [zp[:, slh] for zp in Zprev], CH)
                        nc.scalar.activation(A16[:, slh], ps[0:128, 0:CH],
                                             AF.Identity, bias=bs[l][ot][:])
                    ab = sc.tile([128, N], F16, tag="t1")
                    nc.scalar.activation(ab[:], A16[:], AF.Abs)
                    ex = sc.tile([128, N], F16, tag="t2")
                    nc.scalar.activation(ex[:], ab[:], AF.Exp, scale=-1.0)
                    Ln = sc.tile([128, N], F16, tag="L")
                    nc.scalar.activation(Ln[:], ex[:], AF.Ln, bias=1.0)
                    rl = sc.tile([128, N], F16, tag="t1")
                    nc.vector.tensor_scalar_max(rl[:], A16[:], 0.0)
                    Z = sc.tile([128, N], F16, tag="Z", bufs=3)
                    nc.vector.tensor_add(Z[:], rl[:], Ln[:])
                    d = sc.tile([128, N], F16, tag="t2")
                    nc.vector.tensor_sub(d[:], A16[:], Z[:])
                    St = ap.tile([128, N], F16, tag=f"S{l}_{ot}")
                    nc.scalar.activation(St[:], d[:], AF.Exp)
                    S[(l, ot)] = St
                    Zcur.append(Z)
                Zprev = [z[:] for z in Zcur]

            # ---- D4 / c4 ----------------------------------------------------
            D4, c4 = [], []
            for ot in range(KT):
                Dt4 = ap.tile([128, N], F16, tag=f"D4_{ot}")
                nc.vector.tensor_scalar_mul(Dt4[:], S[(3, ot)][:], w4t[ot][:])
                D4.append(Dt4)
                OmS = sc.tile([128, N], F16, tag="OmS")
                nc.vector.tensor_scalar(OmS[:], S[(3, ot)][:], -1.0, 1.0,
                                        ALU.mult, ALU.add)
                tm = sc.tile([128, N], F16, tag="t1")
                nc.vector.tensor_mul(tm[:], OmS[:], S[(3, ot)][:])
                ct = ap.tile([128, N], F16, tag=f"c4_{ot}")
                nc.vector.tensor_scalar_mul(ct[:], tm[:], w4t[ot][:])
                c4.append(ct)

            # ---- backward D chain + F/E + g_q ------------------------------
            F = {}
            E1 = []
            Dprev = [d[:] for d in D4]
            for l in (2, 1, 0):
                Dcur = []
                for ot in range(KT):
                    lts = [Wn[l + 1][ki][:, ot * 128:(ot + 1) * 128] for ki in range(KT)]
                    Ucp = sc.tile([128, N], F16, tag="Ucp")
                    for h in range(2):
                        slh = slice(h * CH, (h + 1) * CH)
                        ps = psB.tile([128, CH], F32, tag="mm")
                        mm(ps[:], lts, [dp[:, slh] for dp in Dprev], CH)
                        nc.scalar.activation(Ucp[:, slh], ps[0:128, 0:CH], AF.Copy)
                    Dt = sc.tile([128, N], F16, tag="Dt", bufs=3)
                    nc.vector.tensor_mul(Dt[:], Ucp[:], S[(l, ot)][:])
                    if l > 0:
                        Ft = ap.tile([128, N], F16, tag=f"F{l}_{ot}")
                        nc.vector.tensor_sub(Ft[:], Ucp[:], Dt[:])
                        F[(l, ot)] = Ft
                    else:
                        OmS1 = sc.tile([128, N], F16, tag="OmS")
                        nc.vector.tensor_scalar(OmS1[:], S[(0, ot)][:], -1.0, 1.0,
                                                ALU.mult, ALU.add)
                        Et = ap.tile([128, N], F16, tag=f"E1_{ot}")
                        nc.vector.tensor_mul(Et[:], Dt[:], OmS1[:])
                        E1.append(Et)
                    Dcur.append(Dt)
                Dprev = [d[:] for d in Dcur]

            Gcp = hp.tile([IN, N], F32)
            for h in range(2):
                slh = slice(h * CH, (h + 1) * CH)
                psG = psB.tile([128, CH], F32, tag="mm")
                mm(psG[0:IN, 0:CH], [W0n[ki][:] for ki in range(KT)],
                   [dp[:, slh] for dp in Dprev], CH)
                nc.scalar.activation(Gcp[:, slh], psG[0:IN, 0:CH], AF.Copy)

            # transposed g_q per group -> gqT [128, g, 12]
            gqT = hp.tile([128, NG, ND], F32)
            for g in range(NG):
                ptg = psT.tile([128, ND], F32, tag="pt")
                nc.tensor.transpose(ptg[:], Gcp[0:ND, g * 128:(g + 1) * 128],
                                    id32[0:ND, 0:ND])
                nc.vector.tensor_copy(gqT[:, g, :], ptg[:])

            # ---- tangent blocks --------------------------------------------
            Hq = hp.tile([128, NG, ND, ND], F32)   # H[x_j<12, qd_i] -> [p,g,i,j]
            Hm = hp.tile([128, NG, ND, ND], F32)   # 100*H[x_12+j, qd_i]
            Hc = None

            def bca(l, ot, sl):
                return S[(l, ot)][:, sl].unsqueeze(1).broadcast_to((128, ND, T))

            def flat(ts):
                return [t_[:].rearrange("p d t -> p (d t)") for t_ in ts]

            def make_steps(b, Hc_ref):
                """Return the tangent-chain step closures for block b.

                Blocks are emitted pairwise-interleaved so every engine queue
                alternates between two independent chains (fills the bubbles a
                single serial chain leaves on the other engines)."""
                i = b & 1
                off = (b % 4) * T
                g = b // 4
                sl = slice(b * T, (b + 1) * T)
                st = {}

                def w0v(ot):
                    return W0qr[ot][:].rearrange("p (d t) -> p d t", d=ND)

                def tgt(tagbase, **kw):
                    kw.setdefault("bufs", 1)
                    return tg.tile([128, ND, T], F16, tag=f"{tagbase}_{i}",
                                   name=tagbase, **kw)

                def s_zd1():
                    st["Zd1"] = []
                    for ot in range(KT):
                        z = tgt(f"Zd1_{ot}")
                        nc.vector.tensor_mul(z[:], w0v(ot), bca(0, ot, sl))
                        st["Zd1"].append(z)

                def mk_mm(src_key, lW, dst_key, rows=128):
                    def s_mm():
                        pss = []
                        for ot in range(KT if rows == 128 else 1):
                            ps = psB.tile([128, FD], F32, tag="mm", name="ps")
                            if rows == 128:
                                lts = [lW[ki][:, ot * 128:(ot + 1) * 128]
                                       for ki in range(KT)]
                            else:
                                lts = [lW[ki][:] for ki in range(KT)]
                            mm(ps[0:rows, 0:FD], lts, flat(st[src_key]), FD)
                            pss.append(ps)
                        st[dst_key] = pss
                    return s_mm

                def mk_copy(ps_key, dst_key, dtag):
                    def s_copy():
                        st[dst_key] = []
                        for ot in range(KT):
                            cc = tgt(f"{dtag}_{ot}")
                            nc.scalar.activation(cc[:].rearrange("p d t -> p (d t)"),
                                                 st[ps_key][ot][0:128, 0:FD], AF.Copy)
                            st[dst_key].append(cc)
                    return s_copy

                def mk_mul(in_key, coefs, dst_key, dtag=None):
                    def s_mul():
                        st[dst_key] = []
                        for ot in range(KT):
                            z = tgt(f"{dtag or dst_key}_{ot}")
                            nc.vector.tensor_mul(
                                z[:], st[in_key][ot][:],
                                coefs[ot][:, sl].unsqueeze(1).broadcast_to(
                                    (128, ND, T)))
                            st[dst_key].append(z)
                    return s_mul

                def mk_umul_direct(ps_key, coefs, dst_key):
                    def s_mul():
                        st[dst_key] = []
                        for ot in range(KT):
                            z = tgt(f"u2_{ot}")
                            psv = st[ps_key][ot][0:128, 0:FD].rearrange(
                                "p (d t) -> p d t", d=ND)
                            nc.vector.tensor_mul(
                                z[:], psv,
                                coefs[ot][:, sl].unsqueeze(1).broadcast_to(
                                    (128, ND, T)))
                            st[dst_key].append(z)
                    return s_mul

                def mk_tmul(zd_key, coefs, dst_key, use_w0=False):
                    def s_mul():
                        st[dst_key] = []
                        for ot in range(KT):
                            z = tgt(f"t_{ot}")
                            src = w0v(ot) if use_w0 else st[zd_key][ot][:]
                            nc.vector.tensor_mul(
                                z[:], src,
                                coefs[ot][:, sl].unsqueeze(1).broadcast_to(
                                    (128, ND, T)))
                            st[dst_key].append(z)
                    return s_mul

                def mk_add(u_key, t_key, dst_key, dtag, pool_mask=3):
                    def s_add():
                        st[dst_key] = []
                        for ot in range(KT):
                            dd = tgt(f"{dtag}_{ot}")
                            if (pool_mask >> ot) & 1:
                                nc.gpsimd.tensor_add(
                                    dd[:].rearrange("p d t -> p (d t)"),
                                    st[u_key][ot][:].rearrange("p d t -> p (d t)"),
                                    st[t_key][ot][:].rearrange("p d t -> p (d t)"))
                            else:
                                nc.vector.tensor_add(dd[:], st[u_key][ot][:],
                                                     st[t_key][ot][:])
                            st[dst_key].append(dd)
                    return s_add

                def s_hc():
                    if off == 0:
                        Hc_ref[0] = hp.tile([IN, ND, 128], F16, tag="Hc", bufs=2,
                                            name="Hc")
                    nc.scalar.activation(
                        Hc_ref[0][:, :, off:off + T],
                        st["psH"][0][0:IN, 0:FD].rearrange("p (d t) -> p d t", d=ND),
                        AF.Copy)

                def s_hstage():
                    if off != 3 * T:
                        return
                    ptH = psT.tile([128, 288], F16, tag="pt", name="ptH")
                    for dcol in range(ND):
                        nc.tensor.transpose(ptH[:, dcol * IN:(dcol + 1) * IN],
                                            Hc_ref[0][:, dcol, :], id16[0:IN, 0:IN])
                    ptHv = ptH[:, 0:ND * IN].rearrange("p (d k) -> p d k", d=ND)
                    nc.vector.tensor_copy(Hq[:, g, :, :], ptHv[:, :, 0:ND])
                    nc.vector.tensor_scalar_mul(Hm[:, g, :, :], ptHv[:, :, ND:IN],
                                                100.0)

                S2c = [S[(1, 0)], S[(1, 1)]]
                S3c = [S[(2, 0)], S[(2, 1)]]
                F3c = [F[(2, 0)], F[(2, 1)]]
                F2c = [F[(1, 0)], F[(1, 1)]]
                S1c = [S[(0, 0)], S[(0, 1)]]
                return [
                    s_zd1,
                    mk_mm("Zd1", WT[1], "psA"),
                    mk_copy("psA", "c2", "c"),
                    mk_mul("c2", S2c, "Zd2"),
                    mk_mm("Zd2", WT[2], "psB"),
                    mk_copy("psB", "c3", "c"),
                    mk_mul("c3", S3c, "Zd3"),
                    mk_mm("Zd3", WT[3], "psC"),
                    mk_copy("psC", "cY", "c"),
                    mk_mul("cY", c4, "Dd4"),
                    mk_mm("Dd4", Wn[3], "psY3"),
                    mk_copy("psY3", "y3", "y"),
                    mk_mul("y3", S3c, "u3", dtag="u"),
                    mk_tmul("Zd3", F3c, "t3"),
                    mk_add("u3", "t3", "Dd3", "DdA"),
                    mk_mm("Dd3", Wn[2], "psY2"),
                    mk_umul_direct("psY2", S2c, "u2"),
                    mk_tmul("Zd2", F2c, "t2"),
                    mk_add("u2", "t2", "Dd2", "DdB"),
                    mk_mm("Dd2", Wn[1], "psY1"),
                    mk_copy("psY1", "y1", "y"),
                    mk_mul("y1", S1c, "u1", dtag="u"),
                    mk_tmul(None, E1, "t1", use_w0=True),
                    mk_add("u1", "t1", "Dd1", "DdA", pool_mask=1),
                    mk_mm("Dd1", W0n, "psH", rows=IN),
                    s_hc,
                    s_hstage,
                ]

            Hc_refs = [[None] for _ in range(NT // 4)]
            steps_all = [make_steps(b, Hc_refs[b // 4]) for b in range(NT)]
            SKEW = 9
            nsteps = len(steps_all[0])
            for tick in range(nsteps + SKEW * (NT - 1) + 1):
                for b in range(NT):
                    j = tick - SKEW * b
                    if 0 <= j < nsteps:
                        steps_all[b][j]()

            # ---- coriolis + rhs + Neumann solve (all groups fused) ---------
            prod = hp.tile([128, NG, ND, ND], F32, tag="prod", bufs=2)
            nc.vector.tensor_tensor(
                prod[:], Hq[:],
                qd_all[:].unsqueeze(2).broadcast_to((128, NG, ND, ND)), ALU.mult)
            cor = hp.tile([128, NG, ND], F32)
            nc.vector.tensor_reduce(cor[:].unsqueeze(3), prod[:], op=ALU.add, axis=AX.X)
            r = hp.tile([128, NG, ND], F32)
            nc.vector.scalar_tensor_tensor(r[:], cor[:], -1.0, gqT[:],
                                           ALU.mult, ALU.add)
            z = hp.tile([128, NG, ND], F32, tag="z", bufs=2)
            nc.vector.tensor_copy(z[:], r[:])
            for _ in range(3):
                pr = hp.tile([128, NG, ND, ND], F32, tag="prod", bufs=2)
                nc.vector.tensor_tensor(
                    pr[:], Hm[:],
                    z[:].unsqueeze(2).broadcast_to((128, NG, ND, ND)), ALU.mult)
                s_ = hp.tile([128, NG, ND], F32, tag="s", bufs=2)
                nc.vector.tensor_reduce(s_[:].unsqueeze(3), pr[:], op=ALU.add, axis=AX.X)
                zn = hp.tile([128, NG, ND], F32, tag="z", bufs=2)
                nc.vector.scalar_tensor_tensor(zn[:], s_[:], -1.0, r[:],
                                               ALU.mult, ALU.add)
                z = zn
            o = hp.tile([128, NG, ND], F32)
            nc.vector.tensor_scalar_mul(o[:], z[:], 100.0)
            for g in range(NG):
                nc.sync.dma_start(dout[g * 128:(g + 1) * 128, :], o[:, g, :])

    nc.compile()
    return nc


def kernel(**inputs):
    f16 = np.float16
    f32 = np.float32
    q = np.asarray(inputs["q"], f32)
    qdot = np.asarray(inputs["qdot"], f32)
    if "nc" not in _cache:
        _cache["nc"] = build_kernel()
    nc = _cache["nc"]
    W = [np.asarray(inputs[f"W{i}"], f32) for i in range(5)]
    X16 = np.ascontiguousarray(np.concatenate([q, qdot], axis=1)).astype(f16)
    base = {
        "wt0": np.ascontiguousarray(W[0].T).astype(f16),
        "wt1": np.ascontiguousarray(W[1].T).astype(f16),
        "wt2": np.ascontiguousarray(W[2].T).astype(f16),
        "wt3": np.ascontiguousarray(W[3].T).astype(f16),
        "wn1": np.ascontiguousarray(W[1]).astype(f16),
        "wn2": np.ascontiguousarray(W[2]).astype(f16),
        "wn3": np.ascontiguousarray(W[3]).astype(f16),
        "w0n": np.ascontiguousarray(W[0]).astype(f16),
        "w0qr": np.ascontiguousarray(
            np.repeat(W[0][:, ND:].astype(f16), T, axis=1)),
        "b0": inputs["b0"].reshape(H, 1).astype(f32),
        "b1": inputs["b1"].reshape(H, 1).astype(f32),
        "b2": inputs["b2"].reshape(H, 1).astype(f32),
        "b3": inputs["b3"].reshape(H, 1).astype(f32),
        "w4": np.ascontiguousarray(W[4].reshape(H, 1)).astype(f32),
        "id16": np.eye(128, dtype=f16),
        "id32": np.eye(128, dtype=f32),
    }
    in_maps = []
    for c in range(NC):
        m = dict(base)
        m["x16"] = X16[c * N:(c + 1) * N]
        m["qd32"] = np.ascontiguousarray(qdot[c * N:(c + 1) * N])
        in_maps.append(m)
    res = run_bass_kernel_spmd(nc, in_maps, core_ids=list(range(NC)),
                               trace=bool(os.environ.get("LNN_TRACE")))
    _cache["last"] = res
    out = np.concatenate([res.results[c]["qdd"] for c in range(NC)], axis=0)
    return out.astype(f32)
